# revision 2
# baseline (speedup 1.0000x reference)
"""GQA causal attention (B=2, T=2048, D=2048, N=16 q-heads, K=4 kv-heads, H=128)
on 8 Trainium2 NeuronCores.

Sharding: core c -> (batch b = c // 4, kv-head g = c % 4). Each core owns one
batch element and one GQA group (1 kv head + its 4 query heads); the host
pre-transposes activations to [D, T] in bf16, precomputes bf16 RoPE tables,
and sums the 4 per-core bf16 O-projection partials of each batch in f64.

Device design notes:
  - all matmul operands bf16 (same 1 cyc/row PE rate as fp32r, but half the
    DMA bytes and SBUF footprint, and 2x DVE throughput on bf16 ops)
  - scores computed transposed [s, t] per 128-key tile; softmax denominator
    via a ones[128,128]-stationary matmul that accumulates the broadcasted
    row-sum directly (no separate broadcast pass); exp without
    max-subtraction (logits are O(8) here)
  - causal diagonal at 256-query granularity with fully-masked chunks
    skipped; masking by post-exp multiply with 0/1 bf16 masks on DVE
    (2x mode); two 256-wide diagonal chunks share one PSUM tile and a
    single [128,512] exp to halve ACT instruction overhead
  - V projection produced directly transposed (stationary = xkv chunk,
    moving = wv), eliminating the PE transpose pass
  - O-projection PSUM->SBUF staging copies alternate ACT/DVE (Pool cannot
    access PSUM); one [128, 2048] bf16 store per 128-row t-tile
"""

import sys

for _p in ("/opt/trn_rl_repo", "/root/.axon_site/_ro/trn_rl_repo"):
    if _p not in sys.path:
        sys.path.append(_p)

import os

import numpy as np
import ml_dtypes

import concourse.mybir as mybir
import concourse.tile as tile
from concourse import bacc
from concourse.bass_utils import run_bass_kernel_spmd

B, T, D = 2, 2048, 2048
N_HEADS, K_HEADS, H = 16, 4, 128
GH = N_HEADS // K_HEADS          # 4 query heads per core
MIN_TS, MAX_TS = 1.0, 10000.0
NJ = T // 512                    # 4 column chunks of 512
ND = D // 128                    # 16 contraction chunks
SCALE = 1.0 / float(np.sqrt(H))

F32 = mybir.dt.float32
BF16 = mybir.dt.bfloat16
MMDT = BF16

_CACHED_NC = None
_last_in_maps = None


def _build_core_program():
    nc = bacc.Bacc("TRN2", target_bir_lowering=False, debug=False, num_devices=8)

    xqT = nc.dram_tensor("xqT", [D, T], MMDT, kind="ExternalInput").ap()
    xkvT = nc.dram_tensor("xkvT", [D, T], MMDT, kind="ExternalInput").ap()
    wq = nc.dram_tensor("wq", [D, GH * H], MMDT, kind="ExternalInput").ap()
    wk = nc.dram_tensor("wk", [D, H], MMDT, kind="ExternalInput").ap()
    wv = nc.dram_tensor("wv", [D, H], MMDT, kind="ExternalInput").ap()
    wo = nc.dram_tensor("wo", [GH * H, D], MMDT, kind="ExternalInput").ap()
    tabs = nc.dram_tensor("tabs", [128, 4 * T], MMDT, kind="ExternalInput").ap()
    out = nc.dram_tensor("out", [T, D], MMDT, kind="ExternalOutput").ap()

    with tile.TileContext(nc) as tc:
        _emit(tc, nc, xqT, xkvT, wq, wk, wv, wo, tabs, out)
    nc.compile()
    return nc


def _emit(tc, nc, xqT, xkvT, wq, wk, wv, wo, tabs, out):
    from contextlib import ExitStack

    # 3D source views: [partition 128, d-chunk, col]
    xq_src = xqT.rearrange("(kd p) t -> p kd t", p=128)
    xkv_src = xkvT.rearrange("(kd p) t -> p kd t", p=128)
    wq_src = wq.rearrange("(kd p) n -> p kd n", p=128)
    wk_src = wk.rearrange("(kd p) n -> p kd n", p=128)
    wv_src = wv.rearrange("(kd p) n -> p kd n", p=128)
    wo_src = wo.rearrange("(h p) d -> p h d", p=128)
    tab_src = tabs.rearrange("p (i t) -> p i t", i=4)

    with ExitStack() as ctx:
        const = ctx.enter_context(tc.tile_pool(name="const", bufs=1))
        xq_pool = ctx.enter_context(tc.tile_pool(
            name="xq", bufs=int(os.environ.get("V2_XQ", "2"))))
        xkv_pool = ctx.enter_context(tc.tile_pool(name="xkv", bufs=4))
        tab_pool = ctx.enter_context(tc.tile_pool(name="tab", bufs=2))
        qrot_pool = ctx.enter_context(tc.tile_pool(
            name="qrot", bufs=int(os.environ.get("V2_QR", "6"))))
        attnt_pool = ctx.enter_context(tc.tile_pool(
            name="attnt", bufs=int(os.environ.get("V2_AT", "6"))))
        probs_pool = ctx.enter_context(tc.tile_pool(
            name="probs", bufs=int(os.environ.get("V2_PT", "14"))))
        work = ctx.enter_context(tc.tile_pool(name="work", bufs=2))
        osb_pool = ctx.enter_context(tc.tile_pool(
            name="osb", bufs=int(os.environ.get("V2_OSB", "2"))))
        _cfg = os.environ.get("V2_PSUM", "4,2,1,1").split(",")
        _mm, _sc, _at, _lr = (int(x) for x in _cfg)
        psum_mm = ctx.enter_context(
            tc.tile_pool(name="psum_mm", bufs=_mm, space="PSUM"))
        psum_sc = ctx.enter_context(
            tc.tile_pool(name="psum_sc", bufs=_sc, space="PSUM"))
        psum_attn = ctx.enter_context(
            tc.tile_pool(name="psum_attn", bufs=_at, space="PSUM")
        )
        psum_lrow = ctx.enter_context(
            tc.tile_pool(name="psum_lrow", bufs=_lr, space="PSUM")
        )
        _grp = int(os.environ.get("V2_GRP", "16"))

        ones_f = work.tile([128, 128], F32, tag="scratch", name="ones_f")
        nc.vector.memset(ones_f[:], 1.0)
        ones_bf = const.tile([128, 128], MMDT, tag="ones_bf")
        nc.vector.tensor_copy(ones_bf[:], ones_f[:])

        # 0/1 causal masks [128 key-partition, 256 q], keep iff base + t - p >= 0
        mask01 = const.tile([128, 2, 256], MMDT, tag="mask01")
        for i, base in enumerate((0, -128)):
            nc.gpsimd.memset(mask01[:, i, :], 1.0)
            nc.gpsimd.affine_select(
                out=mask01[:, i, :], in_=mask01[:, i, :],
                compare_op=mybir.AluOpType.is_ge,
                fill=0.0, base=base,
                pattern=[[1, 256]], channel_multiplier=-1)

        krot_sb = const.tile([128, T], MMDT, tag="krot")
        v_sb = const.tile([128, T], MMDT, tag="v")
        wq_all = const.tile([128, ND, GH * H], MMDT, tag="wq")
        wk_all = const.tile([128, ND, H], MMDT, tag="wk")
        wv_all = const.tile([128, ND, H], MMDT, tag="wv")
        wo_all = const.tile([128, GH, D], MMDT, tag="wo")

        def rope(dst, src_psum, cc_t, ss_t, w=512):
            # dst = src * cc + swap_halves(src) * ss
            tmp1 = work.tile([128, 512], F32, tag="scratch")
            tmp2 = work.tile([128, 512], F32, tag="scratch")
            nc.vector.tensor_mul(tmp1[0:64, :w], src_psum[64:128, :], ss_t[0:64, :])
            nc.vector.tensor_mul(tmp1[64:128, :w], src_psum[0:64, :], ss_t[64:128, :])
            nc.vector.tensor_mul(tmp2[:, :w], src_psum[:], cc_t[:])
            nc.vector.tensor_add(dst, tmp1[:, :w], tmp2[:, :w])

        for J in range(NJ):
            tsl = slice(J * 512, (J + 1) * 512)

            xq_t = xq_pool.tile([128, ND, 512], MMDT, tag="xq", name=f"xq{J}")
            if J == 0:
                # split the J0 prologue so the first Q matmuls' inputs land
                # first; interleave wq with xq per quarter
                for q in range(4):
                    qs = slice(q * 4, (q + 1) * 4)
                    nc.sync.dma_start(xq_t[:, qs, :], xq_src[:, qs, tsl])
                    nc.sync.dma_start(wq_all[:, qs, :], wq_src[:, qs, :])
            else:
                for q in range(ND // 8):
                    qs = slice(q * 8, (q + 1) * 8)
                    nc.sync.dma_start(xq_t[:, qs, :], xq_src[:, qs, tsl])
            xkv_halves = []
            if J == 0:
                nc.sync.dma_start(wk_all[:], wk_src[:])
                nc.sync.dma_start(wv_all[:], wv_src[:])
            for q in range(2):
                xh = xkv_pool.tile([128, 8, 512], MMDT, tag="xkv",
                                   name=f"xkv{J}_{q}")
                nc.sync.dma_start(xh[:], xkv_src[:, q * 8:(q + 1) * 8, tsl])
                xkv_halves.append(xh)
            tab_t = tab_pool.tile([128, 4, 512], MMDT, tag="tab", name=f"tab{J}")
            nc.sync.dma_start(tab_t[:], tab_src[:, :, tsl])
            ccq_t, ssq_t = tab_t[:, 0, :], tab_t[:, 1, :]
            cck_t, ssk_t = tab_t[:, 2, :], tab_t[:, 3, :]

            # ---- Q projection: sequential heads ----
            qrot = []
            for h in range(GH):
                qps = psum_mm.tile([128, 512], F32, tag="mm", name=f"qps{J}_{h}")
                for kd in range(ND):
                    nc.tensor.matmul(
                        qps[:], wq_all[:, kd, h * 128:(h + 1) * 128],
                        xq_t[:, kd, :], start=(kd == 0), stop=(kd == ND - 1),
                        skip_group_check=True)
                qr = qrot_pool.tile([128, 512], MMDT, tag="qrot",
                                    name=f"qrot{J}_{h}")
                rope(qr[:], qps[:], ccq_t, ssq_t)
                qrot.append(qr)

            # ---- K projection ----
            kps = psum_mm.tile([128, 512], F32, tag="mm")
            for q in range(2):
                xkv_t = xkv_halves[q]
                for kq in range(8):
                    kd = q * 8 + kq
                    nc.tensor.matmul(kps[:], wk_all[:, kd, :], xkv_t[:, kq, :],
                                     start=(kd == 0), stop=(kd == ND - 1),
                                     skip_group_check=True)
            # rope K in two halves: scores k-tiles 4J..4J+1 only need the
            # first 256 columns, shortening the J-boundary critical path
            for kh in range(2):
                ksl = slice(kh * 256, (kh + 1) * 256)
                rope(krot_sb[:, J * 512 + kh * 256:J * 512 + (kh + 1) * 256],
                     kps[:, ksl], cck_t[:, ksl], ssk_t[:, ksl], w=256)

            # ---- V projection, produced directly transposed [s, h]:
            # stationary = xkv d-chunk x s-slice, moving = wv d-chunk ----
            vps = psum_mm.tile([128, 4, 128], F32, tag="mm")
            for st in range(4):
                for q in range(2):
                    xkv_t = xkv_halves[q]
                    for kq in range(8):
                        kd = q * 8 + kq
                        nc.tensor.matmul(
                            vps[:, st, :],
                            xkv_t[:, kq, st * 128:(st + 1) * 128],
                            wv_all[:, kd, :],
                            start=(kd == 0), stop=(kd == ND - 1),
                            skip_group_check=True)
                s_tile = J * 4 + st
                nc.vector.tensor_copy(
                    v_sb[:, s_tile * 128:(s_tile + 1) * 128], vps[:, st, :])

            if J == 0:
                # emitted after the J0 activation loads so it queues behind
                # them; first needed by J0's O-projection
                nc.sync.dma_start(wo_all[:], wo_src[:])

            # ---- SDPA for chunk J, all 4 heads ----
            # emission units: list of sub-chunks (k, qoff, off, mask_idx)
            # sharing one [128,512] sc psum tile and one exp. `off` is the
            # sub-chunk's column offset inside the shared tile; full-width
            # units have a single 512-wide sub-chunk.
            units = [[(k, 0, 0, None)] for k in range(4 * J)]
            units += [
                [(4 * J + 0, 0, 0, 0), (4 * J + 1, 0, 256, 1)],
                [(4 * J + 0, 256, 0, None), (4 * J + 1, 256, 256, None)],
                [(4 * J + 2, 256, 0, 0), (4 * J + 3, 256, 256, 1)],
            ]
            # per-query-half PV/lrow chain order for start/stop flags
            h0_ks = [(k, q) for u in units for (k, q, o, m) in u if q == 0]
            h1_ks = [(k, q) for u in units for (k, q, o, m) in u
                     if q == 256 or len(u) == 1]

            def chain_flags(k, qoff, full, half):
                lst = h0_ks if half == 0 else h1_ks
                idx = lst.index((k, 0 if (full or half == 0) else 256))
                return idx == 0, idx == len(lst) - 1

            attnT = []
            for h in range(GH):
                attn_ps = psum_attn.tile([128, 512], F32, tag="attn")
                lrow_ps = psum_lrow.tile([128, 512], F32, tag="lrow")
                for j0 in range(0, len(units), _grp):
                    grp = units[j0:j0 + _grp]
                    pts = []
                    for unit in grp:
                        full = len(unit) == 1
                        sc = psum_sc.tile([128, 512], F32, tag="sc",
                                          name=f"sc{J}_{h}_{unit[0][0]}_{unit[0][1]}")
                        for (k, qoff, off, mi) in unit:
                            qlen = 512 if full else 256
                            nc.tensor.matmul(
                                sc[:, off:off + qlen],
                                krot_sb[:, k * 128:(k + 1) * 128],
                                qrot[h][:, qoff:qoff + qlen],
                                start=True, stop=True, skip_group_check=True)
                        pt = probs_pool.tile([128, 512], MMDT, tag="probs",
                                             name=f"pt{J}_{h}_{unit[0][0]}")
                        nc.scalar.activation(pt[:], sc[:],
                                             mybir.ActivationFunctionType.Exp,
                                             scale=SCALE)
                        for (k, qoff, off, mi) in unit:
                            if mi is not None:
                                nc.vector.tensor_mul(
                                    pt[:, off:off + 256], pt[:, off:off + 256],
                                    mask01[:, mi, :])
                        pts.append((unit, full, pt))
                    for (unit, full, pt) in pts:
                        for (k, qoff, off, mi) in unit:
                            if full:
                                st0, sp0 = chain_flags(k, 0, True, 0)
                                st1, sp1 = chain_flags(k, 0, True, 1)
                                # both halves share this matmul; chains are
                                # aligned for full-width units
                                nc.tensor.matmul(
                                    attn_ps[:],
                                    v_sb[:, k * 128:(k + 1) * 128],
                                    pt[:], start=st0, stop=(sp0 and sp1),
                                    skip_group_check=True)
                                nc.tensor.matmul(
                                    lrow_ps[:], ones_bf[:], pt[:],
                                    start=st0, stop=(sp0 and sp1),
                                    skip_group_check=True)
                            else:
                                half = 0 if qoff == 0 else 1
                                st, sp = chain_flags(k, qoff, False, half)
                                osl = slice(qoff, qoff + 256)
                                psl = slice(off, off + 256)
                                nc.tensor.matmul(
                                    attn_ps[:, osl],
                                    v_sb[:, k * 128:(k + 1) * 128],
                                    pt[:, psl], start=st, stop=sp,
                                    skip_group_check=True)
                                nc.tensor.matmul(
                                    lrow_ps[:, osl], ones_bf[:], pt[:, psl],
                                    start=st, stop=sp, skip_group_check=True)
                lbc_sb = work.tile([128, 512], F32, tag="scratch")
                nc.vector.reciprocal_approx_fast(lbc_sb[:], lrow_ps[:])
                at = attnt_pool.tile([128, 512], MMDT, tag="attnt")
                nc.vector.tensor_mul(at[:], attn_ps[:], lbc_sb[:])
                attnT.append(at)

            # ---- O projection for chunk J ----
            for tt in range(4):
                csl = slice(tt * 128, (tt + 1) * 128)
                ot = osb_pool.tile([128, D], MMDT, tag="osb",
                                   name=f"ot{J}_{tt}")
                for dj in range(4):
                    ops = psum_mm.tile([128, 512], F32, tag="mm")
                    for h in range(GH):
                        nc.tensor.matmul(
                            ops[:], attnT[h][:, csl],
                            wo_all[:, h, dj * 512:(dj + 1) * 512],
                            start=(h == 0), stop=(h == GH - 1),
                            skip_group_check=True)
                    # Pool cannot touch PSUM; alternate ACT/DVE to balance
                    if dj % 2 == 0:
                        nc.scalar.copy(ot[:, dj * 512:(dj + 1) * 512], ops[:])
                    else:
                        nc.vector.tensor_copy(
                            ot[:, dj * 512:(dj + 1) * 512], ops[:])
                # ACT-triggered HWDGE: output stores stay out of the
                # sync-engine input-prefetch stream
                rsl = slice(J * 512 + tt * 128, J * 512 + (tt + 1) * 128)
                if J == NJ - 1 and tt == 3:
                    # split the final store so the drain tail is shorter
                    for c4 in range(4):
                        nc.scalar.dma_start(
                            out[rsl, c4 * 512:(c4 + 1) * 512],
                            ot[:, c4 * 512:(c4 + 1) * 512])
                else:
                    nc.scalar.dma_start(out[rsl, :], ot[:])


def _rope_tables(positions):
    # positions: [T] int -> cc [128, T] = [cos; cos], ss [128, T] = [-sin; sin]
    half = H // 2
    fraction = 2.0 * np.arange(half, dtype=np.float64) / H
    timescale = MIN_TS * (MAX_TS / MIN_TS) ** fraction
    sinusoid = positions.astype(np.float64)[None, :] / timescale[:, None]
    sin = np.sin(sinusoid)
    cos = np.cos(sinusoid)
    cc = np.concatenate([cos, cos], axis=0)
    ss = np.concatenate([-sin, sin], axis=0)
    return cc, ss


def kernel(Xq, Xkv, q_positions, kv_positions, Wq, Wk, Wv, Wo):
    global _CACHED_NC, _last_in_maps
    if _CACHED_NC is None:
        _CACHED_NC = _build_core_program()
    nc = _CACHED_NC

    BF = ml_dtypes.bfloat16
    Xq = np.asarray(Xq, dtype=np.float32)
    Xkv = np.asarray(Xkv, dtype=np.float32)
    Wq = np.asarray(Wq, dtype=np.float32)
    Wk = np.asarray(Wk, dtype=np.float32)
    Wv = np.asarray(Wv, dtype=np.float32)
    Wo = np.asarray(Wo, dtype=np.float32)
    q_positions = np.asarray(q_positions)
    kv_positions = np.asarray(kv_positions)

    in_maps = []
    for c in range(8):
        b, g = c // 4, c % 4
        ccq, ssq = _rope_tables(q_positions[b])
        cck, ssk = _rope_tables(kv_positions[b])
        tabs = np.ascontiguousarray(
            np.concatenate([ccq, ssq, cck, ssk], axis=1)).astype(BF)
        in_maps.append({
            "xqT": np.ascontiguousarray(Xq[b].T).astype(BF),
            "xkvT": np.ascontiguousarray(Xkv[b].T).astype(BF),
            "wq": np.ascontiguousarray(
                Wq[:, g * GH:(g + 1) * GH, :].reshape(D, GH * H)).astype(BF),
            "wk": np.ascontiguousarray(Wk[:, g, :]).astype(BF),
            "wv": np.ascontiguousarray(Wv[:, g, :]).astype(BF),
            "wo": np.ascontiguousarray(
                Wo[g * GH:(g + 1) * GH].reshape(GH * H, D)).astype(BF),
            "tabs": tabs,
        })

    _last_in_maps = in_maps

    res = run_bass_kernel_spmd(nc, in_maps, list(range(8)))

    outp = np.zeros((B, T, D), dtype=np.float64)
    for c in range(8):
        outp[c // 4] += res.results[c]["out"].astype(np.float64)
    return outp.astype(np.float32)
